# revision 2
# baseline (speedup 1.0000x reference)
"""BlockWiseHistogramEncoder Trainium2 kernel.

Input  x: [16, 1, 512, 512] int32, values in [0, 64).
Output:   [16, 1024, 65] float32. Image is split into 32x32 non-overlapping
16x16 blocks (row-major block order); out[b, l, 1+v] = count(v in block l)/256,
out[b, l, 0] = 0.

Sharding: pure data parallel over batch — 2 batches per core on 8 cores.

Per-core algorithm: SBUF tiles hold 128 blocks (partition = block) with the
block's 256 elements along the free dim (gathered by a strided DMA). For each
of the 64 label values one DVE tensor_scalar(is_equal, reduce-add accum_out)
instruction emits the per-block counts for all 128 blocks at 4x bf16 rate.
ScalarE does the int32->bf16 convert and the final 1/256 scale; GPSIMD zeroes
the bin-0 column, so the vector engine runs only the 64 count ops per tile.
"""
import sys

if "/opt/trn_rl_repo" not in sys.path:
    sys.path.insert(0, "/opt/trn_rl_repo")

import numpy as np

N_CORES = 8
B_PER_CORE = 2
H = W = 512
NC_CLS = 64
BLK = 16
HB = H // BLK          # 32 blocks per side
L = HB * HB            # 1024 blocks
E = BLK * BLK          # 256 elems per block
TILES = L // 128       # 8 tiles of 128 blocks per batch

_nc_cache = None
_run_cache = None


def _build():
    import concourse.bacc as bacc
    import concourse.mybir as mybir
    import concourse.tile as tile

    nc = bacc.Bacc("TRN2", target_bir_lowering=False, debug=False)
    x = nc.dram_tensor("x_in", [B_PER_CORE, H, W], mybir.dt.int32,
                       kind="ExternalInput")
    y = nc.dram_tensor("y_out", [B_PER_CORE, L, NC_CLS + 1], mybir.dt.float32,
                       kind="ExternalOutput")

    with tile.TileContext(nc) as tc:
        with tc.tile_pool(name="io", bufs=4) as io_pool, \
             tc.tile_pool(name="wk", bufs=3) as w_pool, \
             tc.tile_pool(name="hs", bufs=4) as h_pool:
            for b in range(B_PER_CORE):
                xb = x.ap()[b].rearrange("(bh r) (bw c) -> bh bw r c",
                                         r=BLK, c=BLK)
                for t in range(TILES):
                    t_in = io_pool.tile([128, E], mybir.dt.int32)
                    for i in range(4):
                        dst = t_in[32 * i:32 * (i + 1), :].rearrange(
                            "bw (r c) -> bw r c", c=BLK)
                        nc.sync.dma_start(dst, xb[4 * t + i])
                    t_bf = w_pool.tile([128, E], mybir.dt.bfloat16)
                    nc.scalar.copy(t_bf[:], t_in[:])
                    t_h = h_pool.tile([128, NC_CLS + 1], mybir.dt.float32)
                    nc.gpsimd.memset(t_h[:, 0:1], 0.0)
                    t_tr = w_pool.tile([128, E], mybir.dt.bfloat16, tag="tr")
                    for c in range(NC_CLS):
                        nc.vector.tensor_scalar(
                            t_tr[:], t_bf[:], float(c), 0.0,
                            mybir.AluOpType.is_equal, mybir.AluOpType.add,
                            accum_out=t_h[:, c + 1:c + 2])
                    nc.scalar.mul(t_h[:, 1:NC_CLS + 1], t_h[:, 1:NC_CLS + 1],
                                  1.0 / E)
                    nc.sync.dma_start(y.ap()[b, 128 * t:128 * (t + 1)], t_h[:])
    nc.compile()
    return nc


def _get_nc():
    global _nc_cache
    if _nc_cache is None:
        _nc_cache = _build()
    return _nc_cache


def _get_runner():
    """Build the sharded jitted executable once (run_bass_via_pjrt retraces
    per call otherwise)."""
    global _run_cache
    if _run_cache is not None:
        return _run_cache

    import jax
    import jax.numpy as jnp
    from jax.sharding import Mesh, PartitionSpec
    from jax.experimental.shard_map import shard_map
    import concourse.mybir as mybir
    from concourse.bass2jax import (
        _bass_exec_p, install_neuronx_cc_hook, partition_id_tensor)

    nc = _get_nc()
    install_neuronx_cc_hook()

    partition_name = (nc.partition_id_tensor.name
                      if nc.partition_id_tensor else None)
    in_names, out_names, out_avals = [], [], []
    for alloc in nc.m.functions[0].allocations:
        if not isinstance(alloc, mybir.MemoryLocationSet):
            continue
        name = alloc.memorylocations[0].name
        if alloc.kind == "ExternalInput":
            if name != partition_name:
                in_names.append(name)
        elif alloc.kind == "ExternalOutput":
            out_names.append(name)
            out_avals.append(jax.core.ShapedArray(
                tuple(alloc.tensor_shape), mybir.dt.np(alloc.dtype)))
    n_params = len(in_names)
    n_outs = len(out_avals)
    all_in_names = list(in_names) + list(out_names)
    if partition_name is not None:
        all_in_names.append(partition_name)

    def _body(*args):
        operands = list(args)
        if partition_name is not None:
            operands.append(partition_id_tensor())
        outs = _bass_exec_p.bind(
            *operands,
            out_avals=tuple(out_avals),
            in_names=tuple(all_in_names),
            out_names=tuple(out_names),
            lowering_input_output_aliases=(),
            sim_require_finite=True,
            sim_require_nnan=True,
            nc=nc,
        )
        return tuple(outs)

    devices = jax.devices()[:N_CORES]
    mesh = Mesh(np.asarray(devices), ("core",))
    in_specs = (PartitionSpec("core"),) * (n_params + n_outs)
    out_specs = (PartitionSpec("core"),) * n_outs
    donate = tuple(range(n_params, n_params + n_outs))
    sharded = jax.jit(
        shard_map(_body, mesh=mesh, in_specs=in_specs, out_specs=out_specs,
                  check_rep=False),
        donate_argnums=donate, keep_unused=True)

    zero_shapes = [(N_CORES * a.shape[0], *a.shape[1:]) for a in out_avals]
    zero_dtypes = [a.dtype for a in out_avals]

    def run(concat_inputs):
        zeros = [np.zeros(s, d) for s, d in zip(zero_shapes, zero_dtypes)]
        out_arrs = sharded(*concat_inputs, *zeros)
        return {name: np.asarray(out_arrs[i]) for i, name in
                enumerate(out_names)}

    _run_cache = run
    return run


def kernel(x: np.ndarray) -> np.ndarray:
    assert x.shape == (16, 1, H, W) and x.dtype == np.int32, (x.shape, x.dtype)
    run = _get_runner()
    xs = np.ascontiguousarray(x[:, 0])          # [16, 512, 512] = concat of
    out = run([xs])["y_out"]                    # 8 cores' [2, 512, 512]
    return out.reshape(16, L, NC_CLS + 1).astype(np.float32, copy=False)
